# revision 27
# baseline (speedup 1.0000x reference)
"""Multi-head causal attention (B=2, S=2048, D=1024, H=16) on 8 TRN2 NeuronCores.

Sharding: data-parallel over batch (2 groups of 4 cores), tensor-parallel over
heads within a group (4 heads / core).  Each core computes its heads'
Q/K/V projections, attention, and a partial output projection over its
256-wide slice of the concatenated head dim; the host sums the 4 partials per
batch and adds the output bias.

Device-side layout: activations live "feature-major" ([D, S]) so the
contraction dim of every matmul sits on SBUF partitions; the host
pre-transposes q/k/v (free) and pre-slices/transposes the weights.
Scores are computed transposed (ST[k, q]) so softmax'd probabilities come out
in exactly the [k, q] layout the attn@V matmul needs as its moving operand.
Softmax uses no max-subtraction (scores are O(3) here, exp is safe in f32)
and the normalizer comes for free from all-ones columns appended to V:
psum rows 0:64 = sum(exp*V), rows 64:128 = sum(exp) replicated 64x.
Masking is a post-exp multiply by a 0/1 pattern tile (dedup'd, host-built).
"""

import hashlib
import numpy as np
import ml_dtypes

B, S, D, H = 2, 2048, 1024, 16
DK = D // H          # 64
NCORES = 8
GROUP = 4            # cores per batch
HPC = H // GROUP     # heads per core = 4
DL = HPC * DK        # 256 local head dims
NPAIR = HPC // 2     # head pairs per core = 2
KC, QC = 128, 512    # key-chunk (partitions) / query-chunk (free)
NKC, NQC = S // KC, S // QC   # 16, 4
KO = D // 128        # 8 contraction chunks for the projections
BF16 = ml_dtypes.bfloat16

_PROG_CACHE = {}


def _classify_mask(m):
    """m: [S, S] (mask[q, k]; 0 = masked).  Tiles are [KC keys, QC queries] in
    the transposed (ST) orientation.  Returns per-tile class, dedup'd 0/1
    patterns, and column-skip offsets."""
    masked = (m == 0)
    cls = np.zeros((NKC, NQC), np.int8)          # 0 drop, 1 mixed, 2 full-keep
    pid = np.full((NKC, NQC), -1, np.int32)
    c0s = np.zeros((NKC, NQC), np.int32)
    pats = []
    pat_index = {}
    for i in range(NKC):
        for j in range(NQC):
            sub = masked[j * QC:(j + 1) * QC, i * KC:(i + 1) * KC]  # [QC, KC]
            if not sub.any():
                cls[i, j] = 2
                continue
            if sub.all():
                cls[i, j] = 0
                continue
            cls[i, j] = 1
            pat = np.where(sub.T, 0.0, 1.0).astype(np.float32)      # [KC, QC]
            key = hashlib.md5(pat.tobytes()).hexdigest()
            if key not in pat_index:
                pat_index[key] = len(pats)
                pats.append(pat)
            pid[i, j] = pat_index[key]
            col_any_valid = ~sub.T.all(axis=0)                      # [QC]
            nz = np.flatnonzero(col_any_valid)
            c0s[i, j] = int(nz[0]) if len(nz) else QC
    guard = bool((~(m != 0).any(axis=1)).any())   # any fully-masked query row
    return cls, pid, c0s, pats, guard


def _build(cls, pid, c0s, n_pat, guard, use_bq, use_bk, use_bv):
    import concourse.tile as tile
    from concourse import bacc, mybir

    f32 = mybir.dt.float32
    bf16 = mybir.dt.bfloat16
    EXP = mybir.ActivationFunctionType.Exp
    ADD = mybir.AluOpType.add
    MULT = mybir.AluOpType.mult

    nc = bacc.Bacc("TRN2", target_bir_lowering=False, debug=False)

    xqT = nc.dram_tensor("xqT", [D, S], bf16, kind="ExternalInput").ap()
    xkT = nc.dram_tensor("xkT", [D, S], bf16, kind="ExternalInput").ap()
    xvT = nc.dram_tensor("xvT", [D, S], bf16, kind="ExternalInput").ap()
    wqT = nc.dram_tensor("wqT", [D, DL], bf16, kind="ExternalInput").ap()
    wkT = nc.dram_tensor("wkT", [D, DL], bf16, kind="ExternalInput").ap()
    wvT = nc.dram_tensor("wvT", [D, DL], bf16, kind="ExternalInput").ap()
    woT = nc.dram_tensor("woT", [DL, D], bf16, kind="ExternalInput").ap()
    bq_d = nc.dram_tensor("bq", [DL], f32, kind="ExternalInput").ap()
    bk_d = nc.dram_tensor("bk", [DL], f32, kind="ExternalInput").ap()
    bv_d = nc.dram_tensor("bv", [DL], f32, kind="ExternalInput").ap()
    pats_d = nc.dram_tensor("pats", [max(n_pat, 1), KC, QC], bf16,
                            kind="ExternalInput").ap()
    out_d = nc.dram_tensor("out", [S, D], bf16, kind="ExternalOutput").ap()

    kept = [[i for i in range(NKC) if cls[i, j] != 0] for j in range(NQC)]

    import contextlib
    with contextlib.ExitStack() as ctx:
        tc = ctx.enter_context(tile.TileContext(nc))
        singles = ctx.enter_context(tc.tile_pool(name="singles", bufs=1))
        xin = ctx.enter_context(tc.tile_pool(name="xin", bufs=9))
        outp = ctx.enter_context(tc.tile_pool(name="outp", bufs=3))
        ptp = ctx.enter_context(tc.tile_pool(name="ptp", bufs=4))
        lrp = ctx.enter_context(tc.tile_pool(name="lrp", bufs=4))
        # PSUM budget (8 banks): scores "sc" 2x[128,2,512] = 4 banks,
        # proj/oproj "pj" 1x2 = 2 banks, attn accum "at2" 1x2 = 2 banks.
        psA = ctx.enter_context(tc.tile_pool(name="psA", bufs=2, space="PSUM"))
        psB = ctx.enter_context(tc.tile_pool(name="psB", bufs=1, space="PSUM"))
        psC = ctx.enter_context(tc.tile_pool(name="psC", bufs=1, space="PSUM"))

        # --- PE warmup: dummy matmuls on a memset tile while DMAs land ----
        # (HAM needs ~3.4us of sustained PE activity to unthrottle; run it
        # during the initial input DMA so real work starts at full clock.)
        warm = singles.tile([128, 640], bf16, tag="warm")
        nc.vector.memset(warm[:], 0.5)
        wps = psA.tile([128, 2, 512], f32, tag="sc", name="warm_ps")  # noqa
        for w in range(24):
            nc.tensor.matmul(wps[:, w % 2, :], warm[:, 0:128],
                             warm[:, 128:640], start=True, stop=True)

        # --- resident constants (weights on gpsimd queue so the x loads own
        # the sync/HWDGE path) ---------------------------------------------
        wq_sb = singles.tile([128, KO, DL], bf16, tag="wq")
        nc.gpsimd.dma_start(wq_sb[:], wqT.rearrange("(ko p) m -> p ko m", p=128))
        wk_sb = singles.tile([128, KO, DL], bf16, tag="wk")
        nc.gpsimd.dma_start(wk_sb[:], wkT.rearrange("(ko p) m -> p ko m", p=128))
        wv_sb = singles.tile([128, KO, DL], bf16, tag="wv")
        nc.gpsimd.dma_start(wv_sb[:], wvT.rearrange("(ko p) m -> p ko m", p=128))
        wo_sb = singles.tile([128, 2, D], bf16, tag="wo")
        nc.gpsimd.dma_start(wo_sb[:], woT.rearrange("(t p) n -> p t n", p=128))
        if use_bq:
            bq_sb = singles.tile([128, 2], f32, tag="bq")
            nc.sync.dma_start(bq_sb[:], bq_d.rearrange("(m p) -> p m", p=128))
        if use_bk:
            bk_sb = singles.tile([128, 2], f32, tag="bk")
            nc.sync.dma_start(bk_sb[:], bk_d.rearrange("(m p) -> p m", p=128))
        if use_bv:
            bv_sb = singles.tile([128, DL], f32, tag="bv")
            nc.sync.dma_start(bv_sb[:], bv_d.unsqueeze(0).to_broadcast((128, DL)))
        if n_pat > 0:
            pat_sb = singles.tile([128, n_pat, QC], bf16, tag="pats")
            nc.gpsimd.dma_start(pat_sb[:], pats_d.rearrange("n p f -> p n f"))

        # --- persistent activations ---------------------------------------
        QT = singles.tile([128, 2, S], bf16, tag="QT")   # [dk-part, pair, q]
        KT = singles.tile([128, 2, S], bf16, tag="KT")
        AT = singles.tile([128, 2, S], bf16, tag="AT")   # attn out, d-major
        # V extended with ones: [k-part, key-chunk, head, 64 V | 64 ones]
        Vx = singles.tile([128, NKC, HPC, 128], bf16, tag="Vx")
        nc.vector.memset(Vx[:, :, :, DK:128], 1.0)

        # ------------------------------------------------------------------
        xts = [{} for _ in range(NQC)]   # per-step loaded x tiles

        def load_unit(name, src, j):
            def _u():
                t = xin.tile([128, KO, QC], bf16, tag="xin",
                             name=f"x_{name}{j}")
                nc.sync.dma_start(
                    t[:], src.rearrange("(ko p) s -> p ko s", p=128)
                    [:, :, j * QC:(j + 1) * QC])
                xts[j][name] = t
            return _u

        def qkpair_units(name, w_sb, dst, b_sb, j0):
            """Q or K projection for the j-pair (j0, j0+1): each weight chunk
            is loaded once and streams both columns (amortizes LDWEIGHTS)."""
            units = []

            def mm(hold, m, ko0):
                def _u():
                    if "ps" not in hold:
                        hold["ps"] = psB.tile([128, 2, 512], f32, tag="pj",
                                              name=f"ps_{name}{j0}_{m}")
                    ps = hold["ps"]
                    for ko in range(ko0, ko0 + 2):
                        for jj in range(2):
                            nc.tensor.matmul(
                                ps[:, jj, :],
                                w_sb[:, ko, m * 128:(m + 1) * 128],
                                xts[j0 + jj][name][:, ko, :],
                                start=(ko == 0), stop=(ko == KO - 1))
                return _u

            def done(hold, m):
                def _u():
                    ps = hold["ps"]
                    dst_v = dst[:, m, j0 * QC:(j0 + 2) * QC] \
                        .rearrange("p (a b) -> p a b", a=2)
                    if b_sb is not None:
                        for jj in range(2):
                            nc.vector.tensor_scalar_add(
                                dst_v[:, jj, :], ps[:, jj, :], b_sb[:, m:m + 1])
                    else:
                        nc.vector.tensor_copy(out=dst_v, in_=ps[:])
                return _u

            for m in range(2):
                hold = {}
                for ko0 in (0, 2, 4, 6):
                    units.append(mm(hold, m, ko0))
                units.append(done(hold, m))
            return units

        def vproj_units(j):
            units = []
            xt = xts[j]

            def v_mm(hold, s, ko0):
                def _u():
                    if "ps" not in hold:
                        hold["ps"] = psB.tile([128, 2, 512], f32, tag="pj",
                                              name=f"ps_v{j}")
                    ps = hold["ps"]
                    sp = s
                    for ko in range(ko0, ko0 + 4):
                        nc.tensor.matmul(
                            ps[:, s % 2, 0:DL],
                            xt["v"][:, ko, sp * 128:(sp + 1) * 128],
                            wv_sb[:, ko, :],
                            start=(ko == 0), stop=(ko == KO - 1))
                return _u

            def v_done(hold, spp):
                def _u():
                    ps = hold["ps"]
                    for s in range(2):
                        kc = j * 4 + spp * 2 + s
                        src = ps[:, s, 0:DL].rearrange("p (h d) -> p h d",
                                                       h=HPC)
                        dstv = Vx[:, kc, :, 0:DK]
                        if use_bv:
                            nc.vector.tensor_tensor(
                                out=dstv, in0=src,
                                in1=bv_sb.rearrange("p (h d) -> p h d", h=HPC),
                                op=ADD)
                        else:
                            nc.vector.tensor_copy(out=dstv, in_=src)
                return _u

            # two V psum tiles (sp pairs) - each its own hold/group
            for spp in range(2):
                hold = {}
                for s in (spp * 2, spp * 2 + 1):
                    for ko0 in (0, 4):
                        units.append(v_mm(hold, s, ko0))
                units.append(v_done(hold, spp))
            return units

        # ------------------------------------------------------------------
        def attn_units(j):
            """Scores+exp+attnV tile units and epilogue; oproj emitted later."""
            units = []
            st = {}
            klist = kept[j]

            def pair_units(pair):
                n = len(klist)

                def start_pair():
                    st["at2"] = psC.tile([128, 2, 512], f32, tag="at2",
                                         name=f"at{j}_{pair}")
                    st["pt"] = {}

                def score_part(idx, i):
                    """Scores + exp (+mask) for tile idx — runs one step
                    ahead of the attn@V consumer to hide ACT latency."""
                    first = (idx == 0)
                    c0 = 0 if first else int(c0s[i, j])
                    ps = psA.tile([128, 2, 512], f32, tag="sc",
                                  name=f"sc{j}_{pair}_{i}")
                    for hi in range(2):
                        nc.tensor.matmul(
                            ps[:, hi, c0:512],
                            KT[hi * 64:(hi + 1) * 64, pair,
                               i * KC:(i + 1) * KC],
                            QT[hi * 64:(hi + 1) * 64, pair,
                               j * QC + c0:(j + 1) * QC],
                            start=True, stop=True,
                            tile_position=(hi * 64, 0))
                    pt = ptp.tile([128, 2, 512], bf16, tag="pt",
                                  name=f"pt{j}_{pair}_{i}")
                    nc.scalar.activation(out=pt[:, :, c0:512],
                                         in_=ps[:, :, c0:512], func=EXP)
                    if cls[i, j] == 1:
                        patb = pat_sb[:, pid[i, j]:pid[i, j] + 1, c0:512] \
                            .to_broadcast((128, 2, 512 - c0))
                        nc.vector.tensor_tensor(
                            out=pt[:, :, c0:512], in0=pt[:, :, c0:512],
                            in1=patb, op=MULT)
                    st["pt"][idx] = (pt, c0)

                def av_part(idx, i):
                    at2 = st["at2"]
                    pt, c0 = st["pt"].pop(idx)
                    for hi in range(2):
                        nc.tensor.matmul(
                            at2[:, hi, c0:512],
                            Vx[:, i, pair * 2 + hi, :],
                            pt[:, hi, c0:512],
                            start=(idx == 0), stop=(idx == n - 1))

                def tile_unit(idx):
                    def _u():
                        if idx < n:
                            score_part(idx, klist[idx])
                        if idx >= 1:
                            av_part(idx - 1, klist[idx - 1])
                    return _u

                def eplg():
                    def _u():
                        at2 = st["at2"]
                        if guard:
                            nc.vector.tensor_scalar_max(
                                at2[64:128, :, :], at2[64:128, :, :], 1e-30)
                        ls = lrp.tile([64, 2, 512], f32, tag="ls",
                                      name=f"ls{j}_{pair}")
                        nc.vector.tensor_copy(out=ls[:], in_=at2[64:128, :, :])
                        lr = lrp.tile([64, 2, 512], f32, tag="lr",
                                      name=f"lr{j}_{pair}")
                        scr = lrp.tile([64, 2, 512], f32, tag="scr",
                                       name=f"scr{j}_{pair}")
                        nc.vector.reciprocal_approx_accurate(
                            out=lr[:], in_=ls[:], scratch=scr[:])
                        for hi in range(2):
                            nc.vector.tensor_tensor(
                                out=AT[hi * 64:(hi + 1) * 64, pair,
                                       j * QC:(j + 1) * QC],
                                in0=at2[0:64, hi, :], in1=lr[:, hi, :],
                                op=MULT)
                    return _u

                return [start_pair] + \
                    [tile_unit(idx) for idx in range(n + 1)] + \
                    [eplg()]

            if klist:
                for pair in range(NPAIR):
                    units += pair_units(pair)
            else:
                def zero_at():
                    nc.vector.memset(AT[:, :, j * QC:(j + 1) * QC], 0.0)
                units.append(zero_at)
            return units

        def oproj_units(j):
            units = []

            def oproj_mm(hold, sp):
                def _u():
                    s0 = j * QC + sp * 128
                    hold["ps"] = psB.tile([128, 2, 512], f32, tag="pj",
                                          name=f"po{j}_{sp}")
                    ps = hold["ps"]
                    for t in range(2):       # t outer: one lhsT load, 2 MMs
                        for tn in range(2):
                            nc.tensor.matmul(
                                ps[:, tn, :], AT[:, t, s0:s0 + 128],
                                wo_sb[:, t, tn * 512:(tn + 1) * 512],
                                start=(t == 0), stop=(t == 1))
                return _u

            def oproj_out(hold, sp):
                def _u():
                    s0 = j * QC + sp * 128
                    ps = hold["ps"]
                    ot = outp.tile([128, 2, 512], bf16, tag="ot",
                                   name=f"ot{j}_{sp}")
                    if sp % 2 == 0:
                        nc.vector.tensor_copy(out=ot[:], in_=ps[:])
                    else:
                        nc.scalar.copy(out=ot[:], in_=ps[:])
                    nc.gpsimd.dma_start(out_d[s0:s0 + 128, :],
                                        ot.rearrange("p a b -> p (a b)"))
                return _u

            for sp in range(4):
                hold = {}
                units.append(oproj_mm(hold, sp))
                units.append(oproj_out(hold, sp))
            return units

        # --- software-pipelined emission ----------------------------------
        # step j: Qproj(j) first, then attn(j) tiles interleaved with
        # {x loads for j+1, K/V proj(j), oproj(j-1)} as PE filler.
        def interleave(a, p):
            if not a:
                for u in p:
                    u()
                return
            ratio = len(p) / len(a)
            acc, kk = 0.0, 0
            for u in a:
                u()
                acc += ratio
                while acc >= 1.0 and kk < len(p):
                    p[kk]()
                    kk += 1
                    acc -= 1.0
            while kk < len(p):
                p[kk]()
                kk += 1

        for jj in (0, 1):
            for name, src in (("q", xqT), ("k", xkT), ("v", xvT)):
                load_unit(name, src, jj)()
        for j in range(NQC):
            early = []
            if j % 2 == 0:
                for u in qkpair_units("q", wq_sb, QT,
                                      bq_sb if use_bq else None, j):
                    u()
                early += qkpair_units("k", wk_sb, KT,
                                      bk_sb if use_bk else None, j)
            early += vproj_units(j)
            a = attn_units(j)
            cut = (2 * len(a)) // 3
            late = []
            if j + 2 < NQC:
                late += [load_unit(n, s, j + 2)
                         for n, s in (("q", xqT), ("k", xkT), ("v", xvT))]
            if j >= 1:
                late += oproj_units(j - 1)
            interleave(a[:cut], early)
            interleave(a[cut:], late)
        for u in oproj_units(NQC - 1):
            u()

    nc.compile()
    return nc


def _prepare(q, k, v, mask, Wq, bq, Wk, bk, Wv, bv, Wo, bo):
    """Returns (nc, in_maps) — compiled program + per-core input maps."""
    q = np.asarray(q, np.float32)
    k = np.asarray(k, np.float32)
    v = np.asarray(v, np.float32)
    mask_np = np.asarray(mask).reshape(S, S)
    Wq = np.asarray(Wq, np.float32); bq = np.asarray(bq, np.float32)
    Wk = np.asarray(Wk, np.float32); bk = np.asarray(bk, np.float32)
    Wv = np.asarray(Wv, np.float32); bv = np.asarray(bv, np.float32)
    Wo = np.asarray(Wo, np.float32); bo = np.asarray(bo, np.float32)

    cls, pid, c0s, pats, guard = _classify_mask(mask_np)
    n_pat = len(pats)
    use_bq = bool(np.any(bq != 0))
    use_bk = bool(np.any(bk != 0))
    use_bv = bool(np.any(bv != 0))

    key = (cls.tobytes(), pid.tobytes(), c0s.tobytes(), n_pat, guard,
           use_bq, use_bk, use_bv)
    key = hashlib.md5(repr(key).encode()).hexdigest()
    if key not in _PROG_CACHE:
        _PROG_CACHE[key] = _build(cls, pid, c0s, n_pat, guard,
                                  use_bq, use_bk, use_bv)
    nc = _PROG_CACHE[key]

    scale = 1.0 / np.sqrt(np.float32(DK))
    if n_pat:
        pats_arr = np.stack(pats).astype(BF16)
    else:
        pats_arr = np.zeros((1, KC, QC), BF16)

    in_maps = []
    xT = {}
    for b in range(B):
        xT[b] = (q[b].T.astype(BF16), k[b].T.astype(BF16),
                 v[b].T.astype(BF16))
    for c in range(NCORES):
        b, hb = divmod(c, GROUP)
        cols = slice(hb * DL, (hb + 1) * DL)
        qT, kT, vT = xT[b]
        in_maps.append({
            "xqT": qT, "xkT": kT, "xvT": vT,
            "wqT": np.ascontiguousarray((Wq[cols, :] * scale).T).astype(BF16),
            "wkT": np.ascontiguousarray(Wk[cols, :].T).astype(BF16),
            "wvT": np.ascontiguousarray(Wv[cols, :].T).astype(BF16),
            "woT": np.ascontiguousarray(Wo[:, cols].T).astype(BF16),
            "bq": np.ascontiguousarray(bq[cols] * scale, np.float32),
            "bk": np.ascontiguousarray(bk[cols], np.float32),
            "bv": np.ascontiguousarray(bv[cols], np.float32),
            "pats": pats_arr,
        })
    return nc, in_maps


def kernel(q, k, v, mask, Wq, bq, Wk, bk, Wv, bv, Wo, bo):
    from concourse.bass_utils import run_bass_kernel_spmd

    nc, in_maps = _prepare(q, k, v, mask, Wq, bq, Wk, bk, Wv, bv, Wo, bo)
    res = run_bass_kernel_spmd(nc, in_maps, core_ids=list(range(NCORES)))
    bo = np.asarray(bo, np.float32)

    out = np.empty((B, S, D), np.float32)
    for b in range(B):
        acc = res.results[b * GROUP]["out"].astype(np.float32)
        for g in range(1, GROUP):
            acc = acc + res.results[b * GROUP + g]["out"].astype(np.float32)
        out[b] = acc + bo[None, :]
    return out


# revision 32
# speedup vs baseline: 1.0401x; 1.0401x over previous
"""Multi-head causal attention (B=2, S=2048, D=1024, H=16) on 8 TRN2 NeuronCores.

Sharding: data-parallel over batch (2 groups of 4 cores), tensor-parallel over
heads within a group (4 heads / core).  Each core computes its heads'
Q/K/V projections, attention, and a partial output projection over its
256-wide slice of the concatenated head dim; the host sums the 4 partials per
batch and adds the output bias.

Device-side layout: activations live "feature-major" ([D, S]) so the
contraction dim of every matmul sits on SBUF partitions; the host
pre-transposes q/k/v (free) and pre-slices/transposes the weights.
Scores are computed transposed (ST[k, q]) so softmax'd probabilities come out
in exactly the [k, q] layout the attn@V matmul needs as its moving operand.
Softmax uses no max-subtraction (scores are O(3) here, exp is safe in f32)
and the normalizer comes for free from all-ones columns appended to V:
psum rows 0:64 = sum(exp*V), rows 64:128 = sum(exp) replicated 64x.
Masking is a post-exp multiply by a 0/1 pattern tile (dedup'd, host-built).
"""

import hashlib
import numpy as np
import ml_dtypes

B, S, D, H = 2, 2048, 1024, 16
DK = D // H          # 64
NCORES = 8
GROUP = 4            # cores per batch
HPC = H // GROUP     # heads per core = 4
DL = HPC * DK        # 256 local head dims
NPAIR = HPC // 2     # head pairs per core = 2
KC, QC = 128, 512    # key-chunk (partitions) / query-chunk (free)
NKC, NQC = S // KC, S // QC   # 16, 4
KO = D // 128        # 8 contraction chunks for the projections
BF16 = ml_dtypes.bfloat16

_PROG_CACHE = {}


def _classify_mask(m):
    """m: [S, S] (mask[q, k]; 0 = masked).  Tiles are [KC keys, QC queries] in
    the transposed (ST) orientation.  Returns per-tile class, dedup'd 0/1
    patterns, and column-skip offsets."""
    masked = (m == 0)
    cls = np.zeros((NKC, NQC), np.int8)          # 0 drop, 1 mixed, 2 full-keep
    pid = np.full((NKC, NQC), -1, np.int32)
    c0s = np.zeros((NKC, NQC), np.int32)
    pats = []
    pat_index = {}
    for i in range(NKC):
        for j in range(NQC):
            sub = masked[j * QC:(j + 1) * QC, i * KC:(i + 1) * KC]  # [QC, KC]
            if not sub.any():
                cls[i, j] = 2
                continue
            if sub.all():
                cls[i, j] = 0
                continue
            cls[i, j] = 1
            pat = np.where(sub.T, 0.0, 1.0).astype(np.float32)      # [KC, QC]
            key = hashlib.md5(pat.tobytes()).hexdigest()
            if key not in pat_index:
                pat_index[key] = len(pats)
                pats.append(pat)
            pid[i, j] = pat_index[key]
            col_any_valid = ~sub.T.all(axis=0)                      # [QC]
            nz = np.flatnonzero(col_any_valid)
            c0s[i, j] = int(nz[0]) if len(nz) else QC
    guard = bool((~(m != 0).any(axis=1)).any())   # any fully-masked query row
    return cls, pid, c0s, pats, guard


def _build(cls, pid, c0s, n_pat, guard, use_bq, use_bk, use_bv):
    import concourse.tile as tile
    from concourse import bacc, mybir

    f32 = mybir.dt.float32
    bf16 = mybir.dt.bfloat16
    EXP = mybir.ActivationFunctionType.Exp
    ADD = mybir.AluOpType.add
    MULT = mybir.AluOpType.mult

    nc = bacc.Bacc("TRN2", target_bir_lowering=False, debug=False)

    xqT = nc.dram_tensor("xqT", [D, S], bf16, kind="ExternalInput").ap()
    xkT = nc.dram_tensor("xkT", [D, S], bf16, kind="ExternalInput").ap()
    xvT = nc.dram_tensor("xvT", [D, S], bf16, kind="ExternalInput").ap()
    wqT = nc.dram_tensor("wqT", [D, DL], bf16, kind="ExternalInput").ap()
    wkT = nc.dram_tensor("wkT", [D, DL], bf16, kind="ExternalInput").ap()
    wvT = nc.dram_tensor("wvT", [D, DL], bf16, kind="ExternalInput").ap()
    woT = nc.dram_tensor("woT", [DL, D], bf16, kind="ExternalInput").ap()
    bq_d = nc.dram_tensor("bq", [DL], f32, kind="ExternalInput").ap()
    bk_d = nc.dram_tensor("bk", [DL], f32, kind="ExternalInput").ap()
    bv_d = nc.dram_tensor("bv", [DL], f32, kind="ExternalInput").ap()
    pats_d = nc.dram_tensor("pats", [max(n_pat, 1), KC, QC], bf16,
                            kind="ExternalInput").ap()
    out_d = nc.dram_tensor("out", [S, D], bf16, kind="ExternalOutput").ap()

    kept = [[i for i in range(NKC) if cls[i, j] != 0] for j in range(NQC)]

    import contextlib
    with contextlib.ExitStack() as ctx:
        tc = ctx.enter_context(tile.TileContext(nc))
        singles = ctx.enter_context(tc.tile_pool(name="singles", bufs=1))
        xin = ctx.enter_context(tc.tile_pool(name="xin", bufs=9))
        outp = ctx.enter_context(tc.tile_pool(name="outp", bufs=3))
        ptp = ctx.enter_context(tc.tile_pool(name="ptp", bufs=4))
        lrp = ctx.enter_context(tc.tile_pool(name="lrp", bufs=4))
        # PSUM budget (8 banks): scores "sc" 2x[128,2,512] = 4 banks,
        # proj/oproj "pj" 1x2 = 2 banks, attn accum "at2" 1x2 = 2 banks.
        psA = ctx.enter_context(tc.tile_pool(name="psA", bufs=2, space="PSUM"))
        psB = ctx.enter_context(tc.tile_pool(name="psB", bufs=1, space="PSUM"))
        psC = ctx.enter_context(tc.tile_pool(name="psC", bufs=1, space="PSUM"))

        # --- PE warmup: dummy matmuls on a memset tile while DMAs land ----
        # (HAM needs ~3.4us of sustained PE activity to unthrottle; run it
        # during the initial input DMA so real work starts at full clock.)
        warm = singles.tile([128, 640], bf16, tag="warm")
        nc.vector.memset(warm[:], 0.5)
        wps = psA.tile([128, 2, 512], f32, tag="sc", name="warm_ps")  # noqa
        for w in range(56):
            nc.tensor.matmul(wps[:, w % 2, :], warm[:, 0:128],
                             warm[:, 128:640], start=True, stop=True)

        # --- resident constants; DMAs issued later in a hand-tuned order ---
        wq_sb = singles.tile([128, KO, DL], bf16, tag="wq")
        wk_sb = singles.tile([128, KO, DL], bf16, tag="wk")
        wv_sb = singles.tile([128, KO, DL], bf16, tag="wv")
        wo_sb = singles.tile([128, 2, D], bf16, tag="wo")

        def weight_dmas_a():
            nc.gpsimd.dma_start(wq_sb[:],
                                wqT.rearrange("(ko p) m -> p ko m", p=128))
            nc.gpsimd.dma_start(wk_sb[:],
                                wkT.rearrange("(ko p) m -> p ko m", p=128))

        def weight_dmas_b():
            nc.gpsimd.dma_start(wv_sb[:],
                                wvT.rearrange("(ko p) m -> p ko m", p=128))
            nc.gpsimd.dma_start(wo_sb[:],
                                woT.rearrange("(t p) n -> p t n", p=128))
        if use_bq:
            bq_sb = singles.tile([128, 2], f32, tag="bq")
            nc.sync.dma_start(bq_sb[:], bq_d.rearrange("(m p) -> p m", p=128))
        if use_bk:
            bk_sb = singles.tile([128, 2], f32, tag="bk")
            nc.sync.dma_start(bk_sb[:], bk_d.rearrange("(m p) -> p m", p=128))
        if use_bv:
            bv_sb = singles.tile([128, DL], f32, tag="bv")
            nc.sync.dma_start(bv_sb[:], bv_d.unsqueeze(0).to_broadcast((128, DL)))
        if n_pat > 0:
            pat_sb = singles.tile([128, n_pat, QC], bf16, tag="pats")

        def pat_dma():
            if n_pat > 0:
                nc.gpsimd.dma_start(pat_sb[:],
                                    pats_d.rearrange("n p f -> p n f"))

        # --- persistent activations ---------------------------------------
        QT = singles.tile([128, 2, S], bf16, tag="QT")   # [dk-part, pair, q]
        KT = singles.tile([128, 2, S], bf16, tag="KT")
        AT = singles.tile([128, 2, S], bf16, tag="AT")   # attn out, d-major
        # V extended with ones: [k-part, key-chunk, head, 64 V | 64 ones]
        Vx = singles.tile([128, NKC, HPC, 128], bf16, tag="Vx")
        nc.vector.memset(Vx[:, :, :, DK:128], 1.0)

        # ------------------------------------------------------------------
        xts = [{} for _ in range(NQC)]   # per-step loaded x tiles

        def load_unit(name, src, j):
            def _u():
                t = xin.tile([128, KO, QC], bf16, tag="xin",
                             name=f"x_{name}{j}")
                eng = nc.sync if j % 2 == 0 else nc.gpsimd
                eng.dma_start(
                    t[:], src.rearrange("(ko p) s -> p ko s", p=128)
                    [:, :, j * QC:(j + 1) * QC])
                xts[j][name] = t
            return _u

        def qkpair_units(name, w_sb, dst, b_sb, j0):
            """Q or K projection for the j-pair (j0, j0+1): each weight chunk
            is loaded once and streams both columns (amortizes LDWEIGHTS)."""
            units = []

            def mm(hold, m, ko0):
                def _u():
                    if "ps" not in hold:
                        hold["ps"] = psB.tile([128, 2, 512], f32, tag="pj",
                                              name=f"ps_{name}{j0}_{m}")
                    ps = hold["ps"]
                    for ko in range(ko0, ko0 + 2):
                        for jj in range(2):
                            nc.tensor.matmul(
                                ps[:, jj, :],
                                w_sb[:, ko, m * 128:(m + 1) * 128],
                                xts[j0 + jj][name][:, ko, :],
                                start=(ko == 0), stop=(ko == KO - 1))
                return _u

            def done(hold, m):
                def _u():
                    ps = hold["ps"]
                    dst_v = dst[:, m, j0 * QC:(j0 + 2) * QC] \
                        .rearrange("p (a b) -> p a b", a=2)
                    if b_sb is not None:
                        for jj in range(2):
                            nc.vector.tensor_scalar_add(
                                dst_v[:, jj, :], ps[:, jj, :], b_sb[:, m:m + 1])
                    else:
                        nc.vector.tensor_copy(out=dst_v, in_=ps[:])
                return _u

            for m in range(2):
                hold = {}
                for ko0 in (0, 2, 4, 6):
                    units.append(mm(hold, m, ko0))
                units.append(done(hold, m))
            return units

        def vproj_units(j):
            units = []
            xt = xts[j]

            def v_mm(hold, s, ko0):
                def _u():
                    if "ps" not in hold:
                        hold["ps"] = psB.tile([128, 2, 512], f32, tag="pj",
                                              name=f"ps_v{j}")
                    ps = hold["ps"]
                    sp = s
                    for ko in range(ko0, ko0 + 4):
                        nc.tensor.matmul(
                            ps[:, s % 2, 0:DL],
                            xt["v"][:, ko, sp * 128:(sp + 1) * 128],
                            wv_sb[:, ko, :],
                            start=(ko == 0), stop=(ko == KO - 1))
                return _u

            def v_done(hold, spp):
                def _u():
                    ps = hold["ps"]
                    for s in range(2):
                        kc = j * 4 + spp * 2 + s
                        src = ps[:, s, 0:DL].rearrange("p (h d) -> p h d",
                                                       h=HPC)
                        dstv = Vx[:, kc, :, 0:DK]
                        if use_bv:
                            nc.vector.tensor_tensor(
                                out=dstv, in0=src,
                                in1=bv_sb.rearrange("p (h d) -> p h d", h=HPC),
                                op=ADD)
                        else:
                            nc.vector.tensor_copy(out=dstv, in_=src)
                return _u

            # two V psum tiles (sp pairs) - each its own hold/group
            for spp in range(2):
                hold = {}
                for s in (spp * 2, spp * 2 + 1):
                    for ko0 in (0, 4):
                        units.append(v_mm(hold, s, ko0))
                units.append(v_done(hold, spp))
            return units

        # ------------------------------------------------------------------
        def attn_units(j):
            """Scores+exp+attnV tile units and epilogue; oproj emitted later."""
            units = []
            st = {}
            klist = kept[j]

            def pair_units(pair):
                n = len(klist)

                def start_pair():
                    st["at2"] = psC.tile([128, 2, 512], f32, tag="at2",
                                         name=f"at{j}_{pair}")
                    st["pt"] = {}

                def score_part(idx, i):
                    """Scores + exp (+mask) for tile idx — runs one step
                    ahead of the attn@V consumer to hide ACT latency."""
                    first = (idx == 0)
                    c0 = 0 if first else int(c0s[i, j])
                    ps = psA.tile([128, 2, 512], f32, tag="sc",
                                  name=f"sc{j}_{pair}_{i}")
                    for hi in range(2):
                        nc.tensor.matmul(
                            ps[:, hi, c0:512],
                            KT[hi * 64:(hi + 1) * 64, pair,
                               i * KC:(i + 1) * KC],
                            QT[hi * 64:(hi + 1) * 64, pair,
                               j * QC + c0:(j + 1) * QC],
                            start=True, stop=True,
                            tile_position=(hi * 64, 0))
                    pt = ptp.tile([128, 2, 512], bf16, tag="pt",
                                  name=f"pt{j}_{pair}_{i}")
                    nc.scalar.activation(out=pt[:, :, c0:512],
                                         in_=ps[:, :, c0:512], func=EXP)
                    if cls[i, j] == 1:
                        patb = pat_sb[:, pid[i, j]:pid[i, j] + 1, c0:512] \
                            .to_broadcast((128, 2, 512 - c0))
                        nc.vector.tensor_tensor(
                            out=pt[:, :, c0:512], in0=pt[:, :, c0:512],
                            in1=patb, op=MULT)
                    st["pt"][idx] = (pt, c0)

                def av_part(idx, i):
                    at2 = st["at2"]
                    pt, c0 = st["pt"].pop(idx)
                    for hi in range(2):
                        nc.tensor.matmul(
                            at2[:, hi, c0:512],
                            Vx[:, i, pair * 2 + hi, :],
                            pt[:, hi, c0:512],
                            start=(idx == 0), stop=(idx == n - 1))

                def tile_unit(idx):
                    def _u():
                        if idx < n:
                            score_part(idx, klist[idx])
                        if idx >= 1:
                            av_part(idx - 1, klist[idx - 1])
                    return _u

                def eplg():
                    def _u():
                        at2 = st["at2"]
                        if guard:
                            nc.vector.tensor_scalar_max(
                                at2[64:128, :, :], at2[64:128, :, :], 1e-30)
                        ls = lrp.tile([64, 2, 512], f32, tag="ls",
                                      name=f"ls{j}_{pair}")
                        nc.vector.tensor_copy(out=ls[:], in_=at2[64:128, :, :])
                        lr = lrp.tile([64, 2, 512], f32, tag="lr",
                                      name=f"lr{j}_{pair}")
                        scr = lrp.tile([64, 2, 512], f32, tag="scr",
                                       name=f"scr{j}_{pair}")
                        nc.vector.reciprocal_approx_accurate(
                            out=lr[:], in_=ls[:], scratch=scr[:])
                        for hi in range(2):
                            nc.vector.tensor_tensor(
                                out=AT[hi * 64:(hi + 1) * 64, pair,
                                       j * QC:(j + 1) * QC],
                                in0=at2[0:64, hi, :], in1=lr[:, hi, :],
                                op=MULT)
                    return _u

                return [start_pair] + \
                    [tile_unit(idx) for idx in range(n + 1)] + \
                    [eplg()]

            if klist:
                for pair in range(NPAIR):
                    units += pair_units(pair)
            else:
                def zero_at():
                    nc.vector.memset(AT[:, :, j * QC:(j + 1) * QC], 0.0)
                units.append(zero_at)
            return units

        def oproj_units(j):
            units = []

            def oproj_mm(hold, sp):
                def _u():
                    s0 = j * QC + sp * 128
                    hold["ps"] = psB.tile([128, 2, 512], f32, tag="pj",
                                          name=f"po{j}_{sp}")
                    ps = hold["ps"]
                    for t in range(2):       # t outer: one lhsT load, 2 MMs
                        for tn in range(2):
                            nc.tensor.matmul(
                                ps[:, tn, :], AT[:, t, s0:s0 + 128],
                                wo_sb[:, t, tn * 512:(tn + 1) * 512],
                                start=(t == 0), stop=(t == 1))
                return _u

            def oproj_out(hold, sp):
                def _u():
                    s0 = j * QC + sp * 128
                    ps = hold["ps"]
                    ot = outp.tile([128, 2, 512], bf16, tag="ot",
                                   name=f"ot{j}_{sp}")
                    if sp % 2 == 0:
                        nc.vector.tensor_copy(out=ot[:], in_=ps[:])
                    else:
                        nc.scalar.copy(out=ot[:], in_=ps[:])
                    nc.gpsimd.dma_start(out_d[s0:s0 + 128, :],
                                        ot.rearrange("p a b -> p (a b)"))
                return _u

            for sp in range(4):
                hold = {}
                units.append(oproj_mm(hold, sp))
                units.append(oproj_out(hold, sp))
            return units

        # --- software-pipelined emission ----------------------------------
        # step j: Qproj(j) first, then attn(j) tiles interleaved with
        # {x loads for j+1, K/V proj(j), oproj(j-1)} as PE filler.
        def interleave(a, p):
            if not a:
                for u in p:
                    u()
                return
            ratio = len(p) / len(a)
            acc, kk = 0.0, 0
            for u in a:
                u()
                acc += ratio
                while acc >= 1.0 and kk < len(p):
                    p[kk]()
                    kk += 1
                    acc -= 1.0
            while kk < len(p):
                p[kk]()
                kk += 1

        # startup order: q columns first (Q-pair proj is the critical path),
        # weights interleaved on the second queue
        load_unit("q", xqT, 0)()
        load_unit("q", xqT, 1)()
        weight_dmas_a()
        load_unit("k", xkT, 0)()
        load_unit("k", xkT, 1)()
        load_unit("v", xvT, 0)()
        load_unit("v", xvT, 1)()
        weight_dmas_b()
        pat_dma()
        for j in range(NQC):
            early = []
            if j % 2 == 0:
                for u in qkpair_units("q", wq_sb, QT,
                                      bq_sb if use_bq else None, j):
                    u()
                early += qkpair_units("k", wk_sb, KT,
                                      bk_sb if use_bk else None, j)
            early += vproj_units(j)
            a = attn_units(j)
            cut = (2 * len(a)) // 3
            late = []
            if j + 2 < NQC:
                late += [load_unit(n, s, j + 2)
                         for n, s in (("q", xqT), ("k", xkT), ("v", xvT))]
            if j >= 1:
                late += oproj_units(j - 1)
            interleave(a[:cut], early)
            interleave(a[cut:], late)
        for u in oproj_units(NQC - 1):
            u()

    nc.compile()
    return nc


def _prepare(q, k, v, mask, Wq, bq, Wk, bk, Wv, bv, Wo, bo):
    """Returns (nc, in_maps) — compiled program + per-core input maps."""
    q = np.asarray(q, np.float32)
    k = np.asarray(k, np.float32)
    v = np.asarray(v, np.float32)
    mask_np = np.asarray(mask).reshape(S, S)
    Wq = np.asarray(Wq, np.float32); bq = np.asarray(bq, np.float32)
    Wk = np.asarray(Wk, np.float32); bk = np.asarray(bk, np.float32)
    Wv = np.asarray(Wv, np.float32); bv = np.asarray(bv, np.float32)
    Wo = np.asarray(Wo, np.float32); bo = np.asarray(bo, np.float32)

    cls, pid, c0s, pats, guard = _classify_mask(mask_np)
    n_pat = len(pats)
    use_bq = bool(np.any(bq != 0))
    use_bk = bool(np.any(bk != 0))
    use_bv = bool(np.any(bv != 0))

    key = (cls.tobytes(), pid.tobytes(), c0s.tobytes(), n_pat, guard,
           use_bq, use_bk, use_bv)
    key = hashlib.md5(repr(key).encode()).hexdigest()
    if key not in _PROG_CACHE:
        _PROG_CACHE[key] = _build(cls, pid, c0s, n_pat, guard,
                                  use_bq, use_bk, use_bv)
    nc = _PROG_CACHE[key]

    scale = 1.0 / np.sqrt(np.float32(DK))
    if n_pat:
        pats_arr = np.stack(pats).astype(BF16)
    else:
        pats_arr = np.zeros((1, KC, QC), BF16)

    in_maps = []
    xT = {}
    for b in range(B):
        xT[b] = (q[b].T.astype(BF16), k[b].T.astype(BF16),
                 v[b].T.astype(BF16))
    for c in range(NCORES):
        b, hb = divmod(c, GROUP)
        cols = slice(hb * DL, (hb + 1) * DL)
        qT, kT, vT = xT[b]
        in_maps.append({
            "xqT": qT, "xkT": kT, "xvT": vT,
            "wqT": np.ascontiguousarray((Wq[cols, :] * scale).T).astype(BF16),
            "wkT": np.ascontiguousarray(Wk[cols, :].T).astype(BF16),
            "wvT": np.ascontiguousarray(Wv[cols, :].T).astype(BF16),
            "woT": np.ascontiguousarray(Wo[:, cols].T).astype(BF16),
            "bq": np.ascontiguousarray(bq[cols] * scale, np.float32),
            "bk": np.ascontiguousarray(bk[cols], np.float32),
            "bv": np.ascontiguousarray(bv[cols], np.float32),
            "pats": pats_arr,
        })
    return nc, in_maps


def kernel(q, k, v, mask, Wq, bq, Wk, bk, Wv, bv, Wo, bo):
    from concourse.bass_utils import run_bass_kernel_spmd

    nc, in_maps = _prepare(q, k, v, mask, Wq, bq, Wk, bk, Wv, bv, Wo, bo)
    res = run_bass_kernel_spmd(nc, in_maps, core_ids=list(range(NCORES)))
    bo = np.asarray(bo, np.float32)

    out = np.empty((B, S, D), np.float32)
    for b in range(B):
        acc = res.results[b * GROUP]["out"].astype(np.float32)
        for g in range(1, GROUP):
            acc = acc + res.results[b * GROUP + g]["out"].astype(np.float32)
        out[b] = acc + bo[None, :]
    return out


# revision 33
# speedup vs baseline: 1.0501x; 1.0096x over previous
"""Multi-head causal attention (B=2, S=2048, D=1024, H=16) on 8 TRN2 NeuronCores.

Sharding: data-parallel over batch (2 groups of 4 cores), tensor-parallel over
heads within a group (4 heads / core).  Each core computes its heads'
Q/K/V projections, attention, and a partial output projection over its
256-wide slice of the concatenated head dim; the host sums the 4 partials per
batch and adds the output bias.

Device-side layout: activations live "feature-major" ([D, S]) so the
contraction dim of every matmul sits on SBUF partitions; the host
pre-transposes q/k/v (free) and pre-slices/transposes the weights.
Scores are computed transposed (ST[k, q]) so softmax'd probabilities come out
in exactly the [k, q] layout the attn@V matmul needs as its moving operand.
Softmax uses no max-subtraction (scores are O(3) here, exp is safe in f32)
and the normalizer comes for free from all-ones columns appended to V:
psum rows 0:64 = sum(exp*V), rows 64:128 = sum(exp) replicated 64x.
Masking is a post-exp multiply by a 0/1 pattern tile (dedup'd, host-built).
"""

import hashlib
import numpy as np
import ml_dtypes

B, S, D, H = 2, 2048, 1024, 16
DK = D // H          # 64
NCORES = 8
GROUP = 4            # cores per batch
HPC = H // GROUP     # heads per core = 4
DL = HPC * DK        # 256 local head dims
NPAIR = HPC // 2     # head pairs per core = 2
KC, QC = 128, 512    # key-chunk (partitions) / query-chunk (free)
NKC, NQC = S // KC, S // QC   # 16, 4
KO = D // 128        # 8 contraction chunks for the projections
BF16 = ml_dtypes.bfloat16

_PROG_CACHE = {}


def _classify_mask(m):
    """m: [S, S] (mask[q, k]; 0 = masked).  Tiles are [KC keys, QC queries] in
    the transposed (ST) orientation.  Returns per-tile class, dedup'd 0/1
    patterns, and column-skip offsets."""
    masked = (m == 0)
    cls = np.zeros((NKC, NQC), np.int8)          # 0 drop, 1 mixed, 2 full-keep
    pid = np.full((NKC, NQC), -1, np.int32)
    c0s = np.zeros((NKC, NQC), np.int32)
    pats = []
    pat_index = {}
    for i in range(NKC):
        for j in range(NQC):
            sub = masked[j * QC:(j + 1) * QC, i * KC:(i + 1) * KC]  # [QC, KC]
            if not sub.any():
                cls[i, j] = 2
                continue
            if sub.all():
                cls[i, j] = 0
                continue
            cls[i, j] = 1
            pat = np.where(sub.T, 0.0, 1.0).astype(np.float32)      # [KC, QC]
            key = hashlib.md5(pat.tobytes()).hexdigest()
            if key not in pat_index:
                pat_index[key] = len(pats)
                pats.append(pat)
            pid[i, j] = pat_index[key]
            col_any_valid = ~sub.T.all(axis=0)                      # [QC]
            nz = np.flatnonzero(col_any_valid)
            c0s[i, j] = int(nz[0]) if len(nz) else QC
    guard = bool((~(m != 0).any(axis=1)).any())   # any fully-masked query row
    return cls, pid, c0s, pats, guard


def _build(cls, pid, c0s, n_pat, guard, use_bq, use_bk, use_bv):
    import concourse.tile as tile
    from concourse import bacc, mybir

    f32 = mybir.dt.float32
    bf16 = mybir.dt.bfloat16
    EXP = mybir.ActivationFunctionType.Exp
    ADD = mybir.AluOpType.add
    MULT = mybir.AluOpType.mult

    nc = bacc.Bacc("TRN2", target_bir_lowering=False, debug=False)

    xqT = nc.dram_tensor("xqT", [D, S], bf16, kind="ExternalInput").ap()
    xkT = nc.dram_tensor("xkT", [D, S], bf16, kind="ExternalInput").ap()
    xvT = nc.dram_tensor("xvT", [D, S], bf16, kind="ExternalInput").ap()
    wqT = nc.dram_tensor("wqT", [D, DL], bf16, kind="ExternalInput").ap()
    wkT = nc.dram_tensor("wkT", [D, DL], bf16, kind="ExternalInput").ap()
    wvT = nc.dram_tensor("wvT", [D, DL], bf16, kind="ExternalInput").ap()
    woT = nc.dram_tensor("woT", [DL, D], bf16, kind="ExternalInput").ap()
    bq_d = nc.dram_tensor("bq", [DL], f32, kind="ExternalInput").ap()
    bk_d = nc.dram_tensor("bk", [DL], f32, kind="ExternalInput").ap()
    bv_d = nc.dram_tensor("bv", [DL], f32, kind="ExternalInput").ap()
    pats_d = nc.dram_tensor("pats", [max(n_pat, 1), KC, QC], bf16,
                            kind="ExternalInput").ap()
    out_d = nc.dram_tensor("out", [S, D], bf16, kind="ExternalOutput").ap()

    kept = [[i for i in range(NKC) if cls[i, j] != 0] for j in range(NQC)]

    import contextlib
    with contextlib.ExitStack() as ctx:
        tc = ctx.enter_context(tile.TileContext(nc))
        singles = ctx.enter_context(tc.tile_pool(name="singles", bufs=1))
        xin = ctx.enter_context(tc.tile_pool(name="xin", bufs=9))
        outp = ctx.enter_context(tc.tile_pool(name="outp", bufs=3))
        ptp = ctx.enter_context(tc.tile_pool(name="ptp", bufs=4))
        lrp = ctx.enter_context(tc.tile_pool(name="lrp", bufs=4))
        # PSUM budget (8 banks): scores "sc" 2x[128,2,512] = 4 banks,
        # proj/oproj "pj" 1x2 = 2 banks, attn accum "at2" 1x2 = 2 banks.
        psA = ctx.enter_context(tc.tile_pool(name="psA", bufs=2, space="PSUM"))
        psB = ctx.enter_context(tc.tile_pool(name="psB", bufs=1, space="PSUM"))
        psC = ctx.enter_context(tc.tile_pool(name="psC", bufs=1, space="PSUM"))

        # --- PE warmup: dummy matmuls on a memset tile while DMAs land ----
        # (HAM needs ~3.4us of sustained PE activity to unthrottle; run it
        # during the initial input DMA so real work starts at full clock.)
        warm = singles.tile([128, 640], bf16, tag="warm")
        nc.vector.memset(warm[:], 0.5)
        wps = psA.tile([128, 2, 512], f32, tag="sc", name="warm_ps")  # noqa
        for w in range(20):
            nc.tensor.matmul(wps[:, w % 2, :], warm[:, 0:128],
                             warm[:, 128:640], start=True, stop=True)

        # --- resident constants; DMAs issued later in a hand-tuned order ---
        wq_sb = singles.tile([128, KO, DL], bf16, tag="wq")
        wk_sb = singles.tile([128, KO, DL], bf16, tag="wk")
        wv_sb = singles.tile([128, KO, DL], bf16, tag="wv")
        wo_sb = singles.tile([128, 2, D], bf16, tag="wo")

        def weight_dmas_a():
            nc.gpsimd.dma_start(wq_sb[:],
                                wqT.rearrange("(ko p) m -> p ko m", p=128))
            nc.gpsimd.dma_start(wk_sb[:],
                                wkT.rearrange("(ko p) m -> p ko m", p=128))

        def weight_dmas_b():
            nc.gpsimd.dma_start(wv_sb[:],
                                wvT.rearrange("(ko p) m -> p ko m", p=128))
            nc.gpsimd.dma_start(wo_sb[:],
                                woT.rearrange("(t p) n -> p t n", p=128))
        if use_bq:
            bq_sb = singles.tile([128, 2], f32, tag="bq")
            nc.sync.dma_start(bq_sb[:], bq_d.rearrange("(m p) -> p m", p=128))
        if use_bk:
            bk_sb = singles.tile([128, 2], f32, tag="bk")
            nc.sync.dma_start(bk_sb[:], bk_d.rearrange("(m p) -> p m", p=128))
        if use_bv:
            bv_sb = singles.tile([128, DL], f32, tag="bv")
            nc.sync.dma_start(bv_sb[:], bv_d.unsqueeze(0).to_broadcast((128, DL)))
        if n_pat > 0:
            pat_sb = singles.tile([128, n_pat, QC], bf16, tag="pats")

        def pat_dma():
            if n_pat > 0:
                nc.gpsimd.dma_start(pat_sb[:],
                                    pats_d.rearrange("n p f -> p n f"))

        # --- persistent activations ---------------------------------------
        QT = singles.tile([128, 2, S], bf16, tag="QT")   # [dk-part, pair, q]
        KT = singles.tile([128, 2, S], bf16, tag="KT")
        AT = singles.tile([128, 2, S], bf16, tag="AT")   # attn out, d-major
        # V extended with ones: [k-part, key-chunk, head, 64 V | 64 ones]
        Vx = singles.tile([128, NKC, HPC, 128], bf16, tag="Vx")
        nc.vector.memset(Vx[:, :, :, DK:128], 1.0)

        # ------------------------------------------------------------------
        xts = [{} for _ in range(NQC)]   # per-step loaded x tiles

        def load_unit(name, src, j):
            def _u():
                t = xin.tile([128, KO, QC], bf16, tag="xin",
                             name=f"x_{name}{j}")
                eng = nc.sync if j % 2 == 0 else nc.gpsimd
                eng.dma_start(
                    t[:], src.rearrange("(ko p) s -> p ko s", p=128)
                    [:, :, j * QC:(j + 1) * QC])
                xts[j][name] = t
            return _u

        def qkpair_units(name, w_sb, dst, b_sb, j0):
            """Q or K projection for the j-pair (j0, j0+1): each weight chunk
            is loaded once and streams both columns (amortizes LDWEIGHTS)."""
            units = []

            def mm(hold, m, ko0):
                def _u():
                    if "ps" not in hold:
                        hold["ps"] = psB.tile([128, 2, 512], f32, tag="pj",
                                              name=f"ps_{name}{j0}_{m}")
                    ps = hold["ps"]
                    for ko in range(ko0, ko0 + 2):
                        for jj in range(2):
                            nc.tensor.matmul(
                                ps[:, jj, :],
                                w_sb[:, ko, m * 128:(m + 1) * 128],
                                xts[j0 + jj][name][:, ko, :],
                                start=(ko == 0), stop=(ko == KO - 1))
                return _u

            def done(hold, m):
                def _u():
                    ps = hold["ps"]
                    dst_v = dst[:, m, j0 * QC:(j0 + 2) * QC] \
                        .rearrange("p (a b) -> p a b", a=2)
                    if b_sb is not None:
                        for jj in range(2):
                            nc.vector.tensor_scalar_add(
                                dst_v[:, jj, :], ps[:, jj, :], b_sb[:, m:m + 1])
                    else:
                        nc.vector.tensor_copy(out=dst_v, in_=ps[:])
                return _u

            for m in range(2):
                hold = {}
                for ko0 in (0, 2, 4, 6):
                    units.append(mm(hold, m, ko0))
                units.append(done(hold, m))
            return units

        def vproj_units(j):
            units = []
            xt = xts[j]

            def v_mm(hold, s, ko0):
                def _u():
                    if "ps" not in hold:
                        hold["ps"] = psB.tile([128, 2, 512], f32, tag="pj",
                                              name=f"ps_v{j}")
                    ps = hold["ps"]
                    sp = s
                    for ko in range(ko0, ko0 + 4):
                        nc.tensor.matmul(
                            ps[:, s % 2, 0:DL],
                            xt["v"][:, ko, sp * 128:(sp + 1) * 128],
                            wv_sb[:, ko, :],
                            start=(ko == 0), stop=(ko == KO - 1))
                return _u

            def v_done(hold, spp):
                def _u():
                    ps = hold["ps"]
                    for s in range(2):
                        kc = j * 4 + spp * 2 + s
                        src = ps[:, s, 0:DL].rearrange("p (h d) -> p h d",
                                                       h=HPC)
                        dstv = Vx[:, kc, :, 0:DK]
                        if use_bv:
                            nc.vector.tensor_tensor(
                                out=dstv, in0=src,
                                in1=bv_sb.rearrange("p (h d) -> p h d", h=HPC),
                                op=ADD)
                        else:
                            nc.vector.tensor_copy(out=dstv, in_=src)
                return _u

            # two V psum tiles (sp pairs) - each its own hold/group
            for spp in range(2):
                hold = {}
                for s in (spp * 2, spp * 2 + 1):
                    for ko0 in (0, 4):
                        units.append(v_mm(hold, s, ko0))
                units.append(v_done(hold, spp))
            return units

        # ------------------------------------------------------------------
        def attn_units(j):
            """Scores+exp+attnV tile units and epilogue; oproj emitted later."""
            units = []
            st = {}
            klist = kept[j]

            def pair_units(pair):
                n = len(klist)

                def start_pair():
                    st["at2"] = psC.tile([128, 2, 512], f32, tag="at2",
                                         name=f"at{j}_{pair}")
                    st["pt"] = {}

                def score_part(idx, i):
                    """Scores + exp (+mask) for tile idx — runs one step
                    ahead of the attn@V consumer to hide ACT latency."""
                    first = (idx == 0)
                    c0 = 0 if first else int(c0s[i, j])
                    ps = psA.tile([128, 2, 512], f32, tag="sc",
                                  name=f"sc{j}_{pair}_{i}")
                    for hi in range(2):
                        nc.tensor.matmul(
                            ps[:, hi, c0:512],
                            KT[hi * 64:(hi + 1) * 64, pair,
                               i * KC:(i + 1) * KC],
                            QT[hi * 64:(hi + 1) * 64, pair,
                               j * QC + c0:(j + 1) * QC],
                            start=True, stop=True,
                            tile_position=(hi * 64, 0))
                    pt = ptp.tile([128, 2, 512], bf16, tag="pt",
                                  name=f"pt{j}_{pair}_{i}")
                    nc.scalar.activation(out=pt[:, :, c0:512],
                                         in_=ps[:, :, c0:512], func=EXP)
                    if cls[i, j] == 1:
                        patb = pat_sb[:, pid[i, j]:pid[i, j] + 1, c0:512] \
                            .to_broadcast((128, 2, 512 - c0))
                        nc.vector.tensor_tensor(
                            out=pt[:, :, c0:512], in0=pt[:, :, c0:512],
                            in1=patb, op=MULT)
                    st["pt"][idx] = (pt, c0)

                def av_part(idx, i):
                    at2 = st["at2"]
                    pt, c0 = st["pt"].pop(idx)
                    for hi in range(2):
                        nc.tensor.matmul(
                            at2[:, hi, c0:512],
                            Vx[:, i, pair * 2 + hi, :],
                            pt[:, hi, c0:512],
                            start=(idx == 0), stop=(idx == n - 1))

                def tile_unit(idx):
                    def _u():
                        if idx < n:
                            score_part(idx, klist[idx])
                        if idx >= 1:
                            av_part(idx - 1, klist[idx - 1])
                    return _u

                def eplg():
                    def _u():
                        at2 = st["at2"]
                        if guard:
                            nc.vector.tensor_scalar_max(
                                at2[64:128, :, :], at2[64:128, :, :], 1e-30)
                        ls = lrp.tile([64, 2, 512], f32, tag="ls",
                                      name=f"ls{j}_{pair}")
                        nc.vector.tensor_copy(out=ls[:], in_=at2[64:128, :, :])
                        lr = lrp.tile([64, 2, 512], f32, tag="lr",
                                      name=f"lr{j}_{pair}")
                        scr = lrp.tile([64, 2, 512], f32, tag="scr",
                                       name=f"scr{j}_{pair}")
                        nc.vector.reciprocal_approx_accurate(
                            out=lr[:], in_=ls[:], scratch=scr[:])
                        for hi in range(2):
                            nc.vector.tensor_tensor(
                                out=AT[hi * 64:(hi + 1) * 64, pair,
                                       j * QC:(j + 1) * QC],
                                in0=at2[0:64, hi, :], in1=lr[:, hi, :],
                                op=MULT)
                    return _u

                return [start_pair] + \
                    [tile_unit(idx) for idx in range(n + 1)] + \
                    [eplg()]

            if klist:
                for pair in range(NPAIR):
                    units += pair_units(pair)
            else:
                def zero_at():
                    nc.vector.memset(AT[:, :, j * QC:(j + 1) * QC], 0.0)
                units.append(zero_at)
            return units

        def oproj_units(j):
            units = []

            def oproj_mm(hold, sp):
                def _u():
                    s0 = j * QC + sp * 128
                    hold["ps"] = psB.tile([128, 2, 512], f32, tag="pj",
                                          name=f"po{j}_{sp}")
                    ps = hold["ps"]
                    for t in range(2):       # t outer: one lhsT load, 2 MMs
                        for tn in range(2):
                            nc.tensor.matmul(
                                ps[:, tn, :], AT[:, t, s0:s0 + 128],
                                wo_sb[:, t, tn * 512:(tn + 1) * 512],
                                start=(t == 0), stop=(t == 1))
                return _u

            def oproj_out(hold, sp):
                def _u():
                    s0 = j * QC + sp * 128
                    ps = hold["ps"]
                    ot = outp.tile([128, 2, 512], bf16, tag="ot",
                                   name=f"ot{j}_{sp}")
                    if sp % 2 == 0:
                        nc.vector.tensor_copy(out=ot[:], in_=ps[:])
                    else:
                        nc.scalar.copy(out=ot[:], in_=ps[:])
                    nc.gpsimd.dma_start(out_d[s0:s0 + 128, :],
                                        ot.rearrange("p a b -> p (a b)"))
                return _u

            for sp in range(4):
                hold = {}
                units.append(oproj_mm(hold, sp))
                units.append(oproj_out(hold, sp))
            return units

        # --- software-pipelined emission ----------------------------------
        # step j: Qproj(j) first, then attn(j) tiles interleaved with
        # {x loads for j+1, K/V proj(j), oproj(j-1)} as PE filler.
        def interleave(a, p):
            if not a:
                for u in p:
                    u()
                return
            ratio = len(p) / len(a)
            acc, kk = 0.0, 0
            for u in a:
                u()
                acc += ratio
                while acc >= 1.0 and kk < len(p):
                    p[kk]()
                    kk += 1
                    acc -= 1.0
            while kk < len(p):
                p[kk]()
                kk += 1

        # startup order: q columns first (Q-pair proj is the critical path),
        # weights interleaved on the second queue
        load_unit("q", xqT, 0)()
        load_unit("q", xqT, 1)()
        weight_dmas_a()
        load_unit("k", xkT, 0)()
        load_unit("k", xkT, 1)()
        load_unit("v", xvT, 0)()
        load_unit("v", xvT, 1)()
        weight_dmas_b()
        pat_dma()
        for j in range(NQC):
            early = []
            if j % 2 == 0:
                for u in qkpair_units("q", wq_sb, QT,
                                      bq_sb if use_bq else None, j):
                    u()
                early += qkpair_units("k", wk_sb, KT,
                                      bk_sb if use_bk else None, j)
            early += vproj_units(j)
            a = attn_units(j)
            cut = (2 * len(a)) // 3
            late = []
            if j + 2 < NQC:
                late += [load_unit(n, s, j + 2)
                         for n, s in (("q", xqT), ("k", xkT), ("v", xvT))]
            if j >= 1:
                late += oproj_units(j - 1)
            interleave(a[:cut], early)
            interleave(a[cut:], late)
        for u in oproj_units(NQC - 1):
            u()

    nc.compile()
    return nc


def _prepare(q, k, v, mask, Wq, bq, Wk, bk, Wv, bv, Wo, bo):
    """Returns (nc, in_maps) — compiled program + per-core input maps."""
    q = np.asarray(q, np.float32)
    k = np.asarray(k, np.float32)
    v = np.asarray(v, np.float32)
    mask_np = np.asarray(mask).reshape(S, S)
    Wq = np.asarray(Wq, np.float32); bq = np.asarray(bq, np.float32)
    Wk = np.asarray(Wk, np.float32); bk = np.asarray(bk, np.float32)
    Wv = np.asarray(Wv, np.float32); bv = np.asarray(bv, np.float32)
    Wo = np.asarray(Wo, np.float32); bo = np.asarray(bo, np.float32)

    cls, pid, c0s, pats, guard = _classify_mask(mask_np)
    n_pat = len(pats)
    use_bq = bool(np.any(bq != 0))
    use_bk = bool(np.any(bk != 0))
    use_bv = bool(np.any(bv != 0))

    key = (cls.tobytes(), pid.tobytes(), c0s.tobytes(), n_pat, guard,
           use_bq, use_bk, use_bv)
    key = hashlib.md5(repr(key).encode()).hexdigest()
    if key not in _PROG_CACHE:
        _PROG_CACHE[key] = _build(cls, pid, c0s, n_pat, guard,
                                  use_bq, use_bk, use_bv)
    nc = _PROG_CACHE[key]

    scale = 1.0 / np.sqrt(np.float32(DK))
    if n_pat:
        pats_arr = np.stack(pats).astype(BF16)
    else:
        pats_arr = np.zeros((1, KC, QC), BF16)

    in_maps = []
    xT = {}
    for b in range(B):
        xT[b] = (q[b].T.astype(BF16), k[b].T.astype(BF16),
                 v[b].T.astype(BF16))
    for c in range(NCORES):
        b, hb = divmod(c, GROUP)
        cols = slice(hb * DL, (hb + 1) * DL)
        qT, kT, vT = xT[b]
        in_maps.append({
            "xqT": qT, "xkT": kT, "xvT": vT,
            "wqT": np.ascontiguousarray((Wq[cols, :] * scale).T).astype(BF16),
            "wkT": np.ascontiguousarray(Wk[cols, :].T).astype(BF16),
            "wvT": np.ascontiguousarray(Wv[cols, :].T).astype(BF16),
            "woT": np.ascontiguousarray(Wo[:, cols].T).astype(BF16),
            "bq": np.ascontiguousarray(bq[cols] * scale, np.float32),
            "bk": np.ascontiguousarray(bk[cols], np.float32),
            "bv": np.ascontiguousarray(bv[cols], np.float32),
            "pats": pats_arr,
        })
    return nc, in_maps


def kernel(q, k, v, mask, Wq, bq, Wk, bk, Wv, bv, Wo, bo):
    from concourse.bass_utils import run_bass_kernel_spmd

    nc, in_maps = _prepare(q, k, v, mask, Wq, bq, Wk, bk, Wv, bv, Wo, bo)
    res = run_bass_kernel_spmd(nc, in_maps, core_ids=list(range(NCORES)))
    bo = np.asarray(bo, np.float32)

    out = np.empty((B, S, D), np.float32)
    for b in range(B):
        acc = res.results[b * GROUP]["out"].astype(np.float32)
        for g in range(1, GROUP):
            acc = acc + res.results[b * GROUP + g]["out"].astype(np.float32)
        out[b] = acc + bo[None, :]
    return out
